# revision 1
# baseline (speedup 1.0000x reference)
import os
import sys

os.environ.setdefault("JAX_PLATFORMS", "axon,cpu")
if "/opt/trn_rl_repo" not in sys.path:
    sys.path.insert(0, "/opt/trn_rl_repo")

import numpy as np
import ml_dtypes

import concourse.bacc as bacc
import concourse.bass as bass
import concourse.tile as tile
from concourse import masks, mybir
from concourse.bass_utils import run_bass_kernel_spmd

BF16 = mybir.dt.bfloat16
F32 = mybir.dt.float32
AF = mybir.ActivationFunctionType
ALU = mybir.AluOpType
AX = mybir.AxisListType

B, N, M, D = 8, 4096, 64, 1024
H, DH, INNER = 16, 64, 1024
KEYS = N + M  # 4160
EPS = 1e-5
NEG = -1e30
# key chunks for matmul free dim (N-dim) and for row chunks
RCH = [(i * 512, 512) for i in range(8)] + [(N, 64)]
JCH = [(i * 128, 128) for i in range(32)] + [(N, 64)]


def build_bass(debug=False):
    nc = bacc.Bacc("TRN2", debug=True)
    x_d = nc.dram_tensor("x", [N, D], F32, kind="ExternalInput").ap()
    lat_d = nc.dram_tensor("latents", [M, D], F32, kind="ExternalInput").ap()
    mb_d = nc.dram_tensor("maskbias", [1, KEYS], BF16, kind="ExternalInput").ap()
    wq_d = nc.dram_tensor("wq", [D, INNER], BF16, kind="ExternalInput").ap()
    wk_d = nc.dram_tensor("wk", [D, INNER], BF16, kind="ExternalInput").ap()
    wv_d = nc.dram_tensor("wv", [D, INNER], BF16, kind="ExternalInput").ap()
    wout_d = nc.dram_tensor("wout", [INNER, D], BF16, kind="ExternalInput").ap()
    out_d = nc.dram_tensor("out", [M, D], F32, kind="ExternalOutput").ap()
    v_d = nc.dram_tensor("v_scratch", [KEYS, INNER], BF16).ap()

    dbg = {}
    if debug:
        for nm, shp in [("xnT", [128, 8 * KEYS]), ("qT", [128, 512]),
                        ("kT", [128, 8 * KEYS]), ("expb", [128, 8 * KEYS]),
                        ("aoT", [128, 512])]:
            dbg[nm] = nc.dram_tensor(f"dbg_{nm}", shp, BF16, kind="ExternalOutput").ap()

    with tile.TileContext(nc) as tc:
        kernel_body(tc, x_d, lat_d, mb_d, wq_d, wk_d, wv_d, wout_d, out_d, v_d, dbg)
    nc.finalize()
    return nc


def kernel_body(tc, x_d, lat_d, mb_d, wq_d, wk_d, wv_d, wout_d, out_d, v_d, dbg={}):
    nc = tc.nc

    # ---- persistent SBUF singles ----
    xnT, free_xnT = tc.tile([128, 8 * KEYS], BF16, name="xnT")   # d-chunk c at cols c*KEYS
    ident, free_ident = tc.tile([128, 128], BF16, name="ident")
    maskb, free_maskb = tc.tile([1, KEYS], BF16, name="maskb")
    ones1, free_ones1 = tc.tile([1, 128], BF16, name="ones1")
    qT, free_qT = tc.tile([128, 8 * 64], BF16, name="qT")       # pair fp at cols fp*64
    aoT, free_aoT = tc.tile([128, 8 * 64], BF16, name="aoT")    # inner-chunk c at cols c*64
    masks.make_identity(nc, ident[:])
    nc.gpsimd.memset(ones1[:], 1.0)
    nc.sync.dma_start(maskb[:], mb_d[:])

    wq, free_wq = tc.tile([128, 8 * INNER], BF16, name="wq_sb")
    for c in range(8):
        nc.sync.dma_start(wq[:, bass.ts(c, INNER)], wq_d[bass.ts(c, 128), :])

    # ---- phase A: layernorm + transpose -> xnT ----
    pa_in = tc.alloc_tile_pool(name="a_in", bufs=3)
    pa_st = tc.alloc_tile_pool(name="a_stats", bufs=2)
    pa_xn = tc.alloc_tile_pool(name="a_xn", bufs=2)
    pa_ps = tc.alloc_tile_pool(name="a_psum", bufs=4, space="PSUM")

    def ln_rows(src_ap, p, col_off):
        xt = pa_in.tile([p, D], F32, name="xt")
        nc.sync.dma_start(xt[:], src_ap)
        st = pa_st.tile([p, 12], F32, name="st", tag="st")
        nc.vector.bn_stats(st[:, 0:6], xt[:, 0:512])
        nc.vector.bn_stats(st[:, 6:12], xt[:, 512:1024])
        mv = pa_st.tile([p, 2], F32, name="mv", tag="mv")
        nc.vector.bn_aggr(mv[:], st[:])
        # rstd = exp(-0.5*ln(var+eps)) + one Newton step (avoids ACT Sqrt:
        # sqrt lives in a different act table and the table switch breaks
        # walrus codegen "Too many sync wait commands")
        veps = pa_st.tile([p, 1], F32, name="veps", tag="veps")
        nc.vector.tensor_scalar_add(veps[:], mv[:, 1:2], EPS)
        lnv = pa_st.tile([p, 1], F32, name="lnv", tag="lnv")
        nc.scalar.activation(lnv[:], veps[:], AF.Ln)
        r0t = pa_st.tile([p, 1], F32, name="r0t", tag="r0t")
        nc.scalar.activation(r0t[:], lnv[:], AF.Exp, scale=-0.5)
        r2 = pa_st.tile([p, 1], F32, name="r2", tag="r2")
        nc.vector.tensor_mul(r2[:], r0t[:], r0t[:])
        vr2 = pa_st.tile([p, 1], F32, name="vr2", tag="vr2")
        nc.vector.tensor_mul(vr2[:], r2[:], veps[:])
        h = pa_st.tile([p, 1], F32, name="h", tag="h")
        nc.vector.tensor_scalar(h[:], vr2[:], -0.5, 1.5, ALU.mult, ALU.add)
        rstd = pa_st.tile([p, 1], F32, name="rstd", tag="rstd")
        tmp = pa_st.tile([p, 1], F32, name="tmp", tag="tmp")
        nmr = pa_st.tile([p, 1], F32, name="nmr", tag="nmr")
        nc.vector.tensor_mul(rstd[:], r0t[:], h[:])
        nc.vector.tensor_mul(tmp[:], mv[:, 0:1], rstd[:])
        nc.vector.tensor_scalar_mul(nmr[:], tmp[:], -1.0)
        xn = pa_xn.tile([p, D], BF16, name="xn")
        nc.scalar.activation(xn[:], xt[:], AF.Identity, bias=nmr[:], scale=rstd[:])
        for c in range(8):
            pt = pa_ps.tile([128, p], BF16, name="pt")
            nc.tensor.transpose(pt[:], xn[:, bass.ts(c, 128)], ident[0:p, 0:p])
            nc.vector.tensor_copy(xnT[:, c * KEYS + col_off: c * KEYS + col_off + p], pt[:])

    for r in range(32):
        ln_rows(x_d[bass.ts(r, 128), :], 128, r * 128)
    ln_rows(lat_d[:], 64, N)
    if "xnT" in dbg:
        nc.sync.dma_start(dbg["xnT"], xnT[:])
    pa_ps.release()
    pa_xn.release()
    pa_st.release()
    pa_in.release()

    # ---- phase C: q projection, scale folded into wq on host ----
    pc_ps = tc.alloc_tile_pool(name="c_psum", bufs=2, space="PSUM")
    for fp in range(8):
        ps = pc_ps.tile([128, 64], F32, name="qps")
        for dc in range(8):
            nc.tensor.matmul(ps[:], wq[:, dc * INNER + fp * 128: dc * INNER + (fp + 1) * 128],
                             xnT[:, dc * KEYS + N: dc * KEYS + KEYS],
                             start=(dc == 0), stop=(dc == 7))
        nc.vector.tensor_copy(qT[:, bass.ts(fp, 64)], ps[:])
    if "qT" in dbg:
        nc.sync.dma_start(dbg["qT"], qT[:])
    pc_ps.release()
    free_wq()

    # ---- phase B: kv matmul ----
    kT, free_kT = tc.tile([128, 8 * KEYS], BF16, name="kT")  # pair fp at cols fp*KEYS
    wk, free_wk = tc.tile([128, 8 * INNER], BF16, name="wk_sb")
    wv, free_wv = tc.tile([128, 8 * INNER], BF16, name="wv_sb")
    for c in range(8):
        nc.sync.dma_start(wk[:, bass.ts(c, INNER)], wk_d[bass.ts(c, 128), :])
        nc.sync.dma_start(wv[:, bass.ts(c, INNER)], wv_d[bass.ts(c, 128), :])

    pb_kps = tc.alloc_tile_pool(name="b_kpsum", bufs=2, space="PSUM")
    for fp in range(8):
        for (r0, rw) in RCH:
            ps = pb_kps.tile([128, rw], F32, name="kps", tag=f"r{rw}")
            for dc in range(8):
                nc.tensor.matmul(ps[:], wk[:, dc * INNER + fp * 128: dc * INNER + (fp + 1) * 128],
                                 xnT[:, dc * KEYS + r0: dc * KEYS + r0 + rw],
                                 start=(dc == 0), stop=(dc == 7))
            dst = kT[:, fp * KEYS + r0: fp * KEYS + r0 + rw]
            if fp % 2 == 0:
                nc.vector.tensor_copy(dst, ps[:])
            else:
                nc.scalar.activation(dst, ps[:], AF.Copy)
    if "kT" in dbg:
        nc.sync.dma_start(dbg["kT"], kT[:])
    pb_kps.release()

    pb_vps = tc.alloc_tile_pool(name="b_vpsum", bufs=2, space="PSUM")
    pb_vsb = tc.alloc_tile_pool(name="b_vsb", bufs=2)
    for (j0, jw) in JCH:
        vt = pb_vsb.tile([jw, INNER], BF16, name="vt", tag=f"j{jw}")
        for nb in range(2):
            ps = pb_vps.tile([jw, 512], F32, name="vps", tag=f"j{jw}")
            for dc in range(8):
                nc.tensor.matmul(ps[:], xnT[:, dc * KEYS + j0: dc * KEYS + j0 + jw],
                                 wv[:, dc * INNER + nb * 512: dc * INNER + (nb + 1) * 512],
                                 start=(dc == 0), stop=(dc == 7))
            if nb == 0:
                nc.vector.tensor_copy(vt[:, 0:512], ps[:])
            else:
                nc.scalar.activation(vt[:, 512:1024], ps[:], AF.Copy)
        nc.gpsimd.dma_start(v_d[j0:j0 + jw, :], vt[:])
    pb_vps.release()
    pb_vsb.release()
    free_wv()
    free_wk()

    # ---- phase D: sim + softmax -> normalized attn (bf16, reuses xnT buffer) ----
    expb = xnT  # same shape/dtype; WAR deps handled by tile framework
    pd_ps = tc.alloc_tile_pool(name="d_psum", bufs=2, space="PSUM")
    pd_ac = tc.alloc_tile_pool(name="d_acc", bufs=2)
    for fp in range(8):
        acc = pd_ac.tile([128, 12], F32, name="acc", tag="acc")
        for ci, (r0, rw) in enumerate(RCH):
            ps = pd_ps.tile([128, rw], F32, name="sps", tag=f"r{rw}")
            nc.tensor.matmul(ps[:], ones1[:], maskb[:, r0:r0 + rw],
                             start=True, stop=False, skip_group_check=True)
            nc.tensor.matmul(ps[0:64, :], qT[0:64, bass.ts(fp, 64)],
                             kT[0:64, fp * KEYS + r0: fp * KEYS + r0 + rw],
                             start=False, stop=True, skip_group_check=True)
            nc.tensor.matmul(ps[64:128, :], qT[64:128, bass.ts(fp, 64)],
                             kT[64:128, fp * KEYS + r0: fp * KEYS + r0 + rw],
                             start=False, stop=True, skip_group_check=True)
            nc.scalar.activation(expb[:, fp * KEYS + r0: fp * KEYS + r0 + rw], ps[:],
                                 AF.Exp, accum_out=acc[:, ci:ci + 1])
        s = pd_ac.tile([128, 1], F32, name="ssum", tag="ssum")
        rs = pd_ac.tile([128, 1], F32, name="rs", tag="rs")
        nc.vector.tensor_reduce(s[:], acc[:, 0:9], AX.X, ALU.add)
        nc.vector.reciprocal(rs[:], s[:])
        nc.vector.tensor_scalar_mul(expb[:, fp * KEYS: (fp + 1) * KEYS],
                                    expb[:, fp * KEYS: (fp + 1) * KEYS], rs[:])
    if "expb" in dbg:
        nc.sync.dma_start(dbg["expb"], expb[:])
    pd_ps.release()
    pd_ac.release()
    free_kT()

    # ---- phase E: attn @ v ----
    wout, free_wout = tc.tile([128, 8 * D], BF16, name="wout_sb")
    for c in range(8):
        nc.sync.dma_start(wout[:, bass.ts(c, D)], wout_d[bass.ts(c, 128), :])
    pe_ops = tc.alloc_tile_pool(name="e_opsum", bufs=1, space="PSUM")
    pe_tps = tc.alloc_tile_pool(name="e_tpsum", bufs=3, space="PSUM")
    pe_at = tc.alloc_tile_pool(name="e_attnT", bufs=4)
    pe_v = tc.alloc_tile_pool(name="e_v", bufs=2)
    nj = len(JCH)
    # one accumulation group per PSUM bank at a time -> two waves of 4 pairs
    for w in range(2):
        ops = [pe_ops.tile([128, 128], F32, name=f"ops{w}_{i}", tag=f"ops{i}")
               for i in range(4)]
        for ji, (j0, jw) in enumerate(JCH):
            vt = pe_v.tile([jw, 512], BF16, name="vte", tag="vte")
            nc.sync.dma_start(vt[:], v_d[j0:j0 + jw, w * 512:(w + 1) * 512])
            for fi in range(4):
                fp = w * 4 + fi
                tp = pe_tps.tile([jw, 128], BF16, name="tp", tag="tp")
                nc.tensor.transpose(tp[:], expb[:, fp * KEYS + j0: fp * KEYS + j0 + jw],
                                    ident[:])
                at = pe_at.tile([jw, 128], BF16, name="at", tag="at")
                nc.vector.tensor_copy(at[:], tp[:])
                nc.tensor.matmul(ops[fi], vt[:, bass.ts(fi, 128)], at[:],
                                 start=(ji == 0), stop=(ji == nj - 1),
                                 skip_group_check=True)
        for fi in range(4):
            fp = w * 4 + fi
            nc.vector.tensor_copy(aoT[0:64, bass.ts(fp, 64)], ops[fi][0:64, 0:64])
            nc.vector.tensor_copy(aoT[64:128, bass.ts(fp, 64)], ops[fi][64:128, 64:128])
    if "aoT" in dbg:
        nc.sync.dma_start(dbg["aoT"], aoT[:])
    pe_v.release()
    pe_at.release()
    pe_tps.release()
    pe_ops.release()

    # ---- phase F: out projection + final LN ----
    pf_ps = tc.alloc_tile_pool(name="f_psum", bufs=2, space="PSUM")
    pf_sb = tc.alloc_tile_pool(name="f_sb", bufs=1)
    pss = []
    for nb in range(2):
        ps = pf_ps.tile([64, 512], F32, name="fps")
        for c in range(8):
            nc.tensor.matmul(ps[:], aoT[:, bass.ts(c, 64)],
                             wout[:, c * D + nb * 512: c * D + (nb + 1) * 512],
                             start=(c == 0), stop=(c == 7))
        pss.append(ps)
    st = pf_sb.tile([64, 12], F32, name="fst", tag="st")
    nc.vector.bn_stats(st[:, 0:6], pss[0][:])
    nc.vector.bn_stats(st[:, 6:12], pss[1][:])
    mv = pf_sb.tile([64, 2], F32, name="fmv", tag="mv")
    nc.vector.bn_aggr(mv[:], st[:])
    veps = pf_sb.tile([64, 1], F32, name="fveps", tag="veps")
    nc.vector.tensor_scalar_add(veps[:], mv[:, 1:2], EPS)
    lnv = pf_sb.tile([64, 1], F32, name="flnv", tag="lnv")
    nc.scalar.activation(lnv[:], veps[:], AF.Ln)
    r0t = pf_sb.tile([64, 1], F32, name="fr0t", tag="r0t")
    nc.scalar.activation(r0t[:], lnv[:], AF.Exp, scale=-0.5)
    r2 = pf_sb.tile([64, 1], F32, name="fr2", tag="r2")
    nc.vector.tensor_mul(r2[:], r0t[:], r0t[:])
    vr2 = pf_sb.tile([64, 1], F32, name="fvr2", tag="vr2")
    nc.vector.tensor_mul(vr2[:], r2[:], veps[:])
    h = pf_sb.tile([64, 1], F32, name="fh", tag="h")
    nc.vector.tensor_scalar(h[:], vr2[:], -0.5, 1.5, ALU.mult, ALU.add)
    rstd = pf_sb.tile([64, 1], F32, name="frstd", tag="rstd")
    tmp = pf_sb.tile([64, 1], F32, name="ftmp", tag="tmp")
    nmr = pf_sb.tile([64, 1], F32, name="fnmr", tag="nmr")
    nc.vector.tensor_mul(rstd[:], r0t[:], h[:])
    nc.vector.tensor_mul(tmp[:], mv[:, 0:1], rstd[:])
    nc.vector.tensor_scalar_mul(nmr[:], tmp[:], -1.0)
    ot = pf_sb.tile([64, D], F32, name="ot", tag="ot")
    nc.scalar.activation(ot[:, 0:512], pss[0][:], AF.Identity, bias=nmr[:], scale=rstd[:])
    nc.scalar.activation(ot[:, 512:1024], pss[1][:], AF.Identity, bias=nmr[:], scale=rstd[:])
    nc.sync.dma_start(out_d[:], ot[:])
    pf_ps.release()
    pf_sb.release()
    free_wout()
    free_aoT()
    free_qT()
    free_ones1()
    free_maskb()
    free_ident()
    free_xnT()


def prep_inputs(x, latents, mask, ln_x_g, ln_x_b, ln_l_g, ln_l_b, Wq, Wkv, Wout,
                ln_o_g, ln_o_b):
    for g in (ln_x_g, ln_l_g, ln_o_g):
        assert np.allclose(np.asarray(g), 1.0)
    for b in (ln_x_b, ln_l_b, ln_o_b):
        assert np.allclose(np.asarray(b), 0.0)
    bf = ml_dtypes.bfloat16
    wq = (np.asarray(Wq, np.float32) * (DH ** -0.5)).astype(bf)
    wk = np.ascontiguousarray(np.asarray(Wkv, np.float32)[:, :INNER]).astype(bf)
    wv = np.ascontiguousarray(np.asarray(Wkv, np.float32)[:, INNER:]).astype(bf)
    wout = np.asarray(Wout, np.float32).astype(bf)
    mb = np.where(np.asarray(mask), np.float32(0.0), np.float32(NEG))
    mb = np.concatenate([mb, np.zeros((B, M), np.float32)], axis=1).astype(bf)
    x = np.ascontiguousarray(np.asarray(x, np.float32))
    latents = np.ascontiguousarray(np.asarray(latents, np.float32))
    in_maps = []
    for i in range(B):
        in_maps.append({
            "x": x[i], "latents": latents[i],
            "maskbias": np.ascontiguousarray(mb[i][None, :]),
            "wq": wq, "wk": wk, "wv": wv, "wout": wout,
        })
    return in_maps


def kernel_with_results(**inputs):
    nc = build_bass()
    in_maps = prep_inputs(**inputs)
    res = run_bass_kernel_spmd(nc, in_maps, list(range(B)))
    out = np.stack([np.asarray(res.results[i]["out"], np.float32) for i in range(B)])
    return out, res


def kernel(**inputs) -> np.ndarray:
    return kernel_with_results(**inputs)[0]


if __name__ == "__main__":
    nc = build_bass()
    print("built ok")

